# revision 1
# baseline (speedup 1.0000x reference)
"""AttentionBlock (GroupNorm -> qkv -> attention -> proj -> residual) on 8 TRN2 cores.

Data-parallel over batch: B=8 samples, one per NeuronCore; no collectives.

Per-core layout strategy (C=512 channels, T=4096 tokens):
  - x, h, q, k held as [C, T] (channels on partitions, 4 tiles of [128, T]).
  - v produced directly transposed: vT [T, C] (32 tiles of [128, 512]).
  - scores computed TRANSPOSED: ST[s, t] = sum_c k[c,s] q[c,t]  (lhsT=k, rhs=q),
    so softmax's reduction axis (s) lands on partitions -> denominator comes
    from a ones-vector matmul on the PE, and exp(ST) evacuated by the scalar
    engine PSUM->SBUF directly yields P^T tiles for the PV matmul. No max
    subtraction (|S| <~ 6 for unit-variance GN'd inputs, exp is safe in f32),
    no transposes anywhere in the hot loop.
  - PV: attn[c, t] = sum_s vT[s,c] P^T[s,t]  (lhsT=vT, rhs=PT).
  - proj: out[o, t] = sum_c projT[c,o] attn[c,t], epilogue fuses +bias +residual.

Matmul inputs are bf16 (full TensorE rate); all accumulation is f32 in PSUM.
Weight transposes / q,k pre-scaling by C**-0.25 / v-bias folding
(softmax rows sum to 1 => v bias contributes proj_w @ b_v + proj_b) are done
on the host in numpy - O(C^2) one-time prep.
"""

import os
import sys

for _p in ("/opt/trn_rl_repo", "/opt/pypackages"):
    if os.path.isdir(_p) and _p not in sys.path:
        sys.path.insert(0, _p)

import numpy as np
import ml_dtypes

import json as _json

import concourse.bass as bass
import concourse.tile as tile
from concourse import mybir
from concourse.bass_utils import run_bass_kernel_spmd

# Walrus's codegen (setupSyncWait) encodes at most ONE sync wait on a DMA
# instruction and errors out ("Too many sync wait commands") instead of
# splitting. Tile's scheduler freely attaches several waits. This pass hoists
# excess waits into standalone EventSemaphore instructions on the same engine
# immediately before the offending instruction — semantically identical (the
# engine's sequencer evaluates them in stream order before issuing it).
_WAIT_LIMITS = {"DMACopy": 1}
_WAIT_LIMIT_DEFAULT = 1


def _legalize_sync_waits(raw: bytes) -> bytes:
    d = _json.loads(raw)
    n_hoisted = 0
    for fn in d.get("functions", []):
        for blk in fn.get("blocks", []):
            out = []
            for inst in blk["instructions"]:
                si = inst.get("sync_info")
                waits = (si or {}).get("on_wait") or []
                limit = _WAIT_LIMITS.get(inst.get("opcode"), _WAIT_LIMIT_DEFAULT)
                if len(waits) > limit and inst.get("engine") not in (
                        None, "Unassigned"):
                    keep = waits[-limit:]
                    hoist = waits[:-limit]
                    for j, w in enumerate(hoist):
                        out.append({
                            "debug": inst.get("debug", 0),
                            "engine": inst["engine"],
                            "ins": [], "outs": [],
                            "name": f"{inst['name']}-hw{j}",
                            "opcode": "EventSemaphore",
                            "sync_info": {"on_update": [], "on_wait": [w]},
                        })
                        n_hoisted += 1
                    si["on_wait"] = keep
                out.append(inst)
            blk["instructions"] = out
    if n_hoisted:
        d.setdefault("attributes", {})
    return _json.dumps(d).encode()


def _install_wait_legalizer(nc):
    orig = nc.to_json_bytes

    def patched():
        return _legalize_sync_waits(orig())

    nc.to_json_bytes = patched

F32 = mybir.dt.float32
BF16 = mybir.dt.bfloat16
AL = mybir.AluOpType
AF = mybir.ActivationFunctionType

C = 512
G = 32          # groupnorm groups
NCT = C // 128  # 4 channel tiles
EPS = 1e-5
TCH = 512       # t-chunk width


def build_graph(T, n_cores=8):
    NT = T // 128
    NCH = T // TCH
    nc = bass.Bass("TRN2", target_bir_lowering=False, debug=False,
                   num_devices=n_cores)

    x_d = nc.dram_tensor("x", [C, T], F32, kind="ExternalInput").ap()
    wqkvt_d = nc.dram_tensor("wqkvt", [C, 3 * C], BF16, kind="ExternalInput").ap()
    wprojt_d = nc.dram_tensor("wprojt", [C, C], BF16, kind="ExternalInput").ap()
    bqk_d = nc.dram_tensor("bqk", [2 * C, 1], F32, kind="ExternalInput").ap()
    bout_d = nc.dram_tensor("bout", [C, 1], F32, kind="ExternalInput").ap()
    gnw_d = nc.dram_tensor("gnw", [C, 1], F32, kind="ExternalInput").ap()
    gnb_d = nc.dram_tensor("gnb", [C, 1], F32, kind="ExternalInput").ap()
    out_d = nc.dram_tensor("out", [C, T], F32, kind="ExternalOutput").ap()

    with tile.TileContext(nc) as tc:
        with (
            tc.tile_pool(name="singles", bufs=1) as sing,
            tc.tile_pool(name="persist", bufs=1) as pers,
        ):
            # ---- weights & constants (resident whole kernel) ----
            wqkvt_sb = []
            for i in range(NCT):
                w = sing.tile([128, 3 * C], BF16, name=f"wqkvt{i}", tag=f"wqkvt{i}")
                nc.gpsimd.dma_start(w, wqkvt_d[i * 128:(i + 1) * 128, :])
                wqkvt_sb.append(w)
            wprojt_sb = []
            for i in range(NCT):
                w = sing.tile([128, C], BF16, name=f"wprojt{i}", tag=f"wprojt{i}")
                nc.gpsimd.dma_start(w, wprojt_d[i * 128:(i + 1) * 128, :])
                wprojt_sb.append(w)
            bout_sb = []
            for i in range(NCT):
                b = sing.tile([128, 1], F32, name=f"bout{i}", tag=f"bout{i}")
                nc.gpsimd.dma_start(b, bout_d[i * 128:(i + 1) * 128, :])
                bout_sb.append(b)
            ones_sb = sing.tile([128, 1], BF16, name="ones", tag="ones")
            nc.vector.memset(ones_sb, 1.0)
            # groupnorm per-channel affine coefs (computed in phase 1)
            A_sb = [sing.tile([128, 1], F32, name=f"gnA{i}", tag=f"gnA{i}")
                    for i in range(NCT)]
            B_sb = [sing.tile([128, 1], F32, name=f"gnB{i}", tag=f"gnB{i}")
                    for i in range(NCT)]

            # ---- persistent activations ----
            q_sb = [pers.tile([128, T], BF16, name=f"q{i}", tag=f"q{i}")
                    for i in range(NCT)]
            k_sb = [pers.tile([128, T], BF16, name=f"k{i}", tag=f"k{i}")
                    for i in range(NCT)]
            vT_sb = [pers.tile([128, C], BF16, name=f"vt{s}", tag=f"vt{s}")
                     for s in range(NT)]

            with tc.tile_pool(name="ph", bufs=1) as ph:
                h_sb = [ph.tile([128, T], BF16, name=f"h{i}", tag=f"h{i}")
                        for i in range(NCT)]

                # ================= phase 1: GroupNorm =================
                # Cross-partition group reductions/broadcasts are done with
                # DRAM round-trip DMAs (strided gather APs) -- no fp32
                # matmuls (walrus chokes on their sync waits), full f32
                # precision.
                with (
                    tc.tile_pool(name="px", bufs=2) as px,
                    tc.tile_pool(name="gns", bufs=2) as gns,
                    tc.tile_pool(name="gnd", bufs=1, space="DRAM") as gnd,
                ):
                    nbn = T // 512
                    gscr = gnd.tile([C, 2], F32, name="gscr", tag="gscr")
                    for ci in range(NCT):
                        xt = px.tile([128, T], F32, name="gn_x", tag="x")
                        nc.gpsimd.dma_start(xt, x_d[ci * 128:(ci + 1) * 128, :])
                        bns = gns.tile([128, nbn, 6], F32, name="bns", tag="bns")
                        x3 = xt.rearrange("p (n f) -> p n f", f=512)
                        for j in range(nbn):
                            nc.vector.bn_stats(bns[:, j, :], x3[:, j, :])
                        mv = gns.tile([128, 2], F32, name="mv", tag="mv")
                        nc.vector.bn_aggr(mv, bns)
                        # mv[:,1] <- E[x^2] = mu^2 + var
                        nc.vector.scalar_tensor_tensor(
                            mv[:, 1:2], in0=mv[:, 0:1], scalar=mv[:, 0:1],
                            in1=mv[:, 1:2], op0=AL.mult, op1=AL.add)
                        nc.gpsimd.dma_start(gscr[ci * 128:(ci + 1) * 128, :], mv)
                    # reshape-load per-channel stats: one group = 16 rows of
                    # [ch, 2] = 32 contiguous floats -> [G, 32] tile
                    gsize = C // G
                    gst = gns.tile([G, 2 * gsize], F32, name="gst", tag="gst")
                    gather = bass.AP(tensor=gscr.tensor, offset=gscr.offset,
                                     ap=[[2 * gsize, G], [1, 2 * gsize]])
                    nc.gpsimd.dma_start(gst, gather)
                    gv = gns.tile([G, 2], F32, name="gv", tag="gv")
                    # strided view [g, stat, ch] then reduce over channels
                    nc.vector.reduce_sum(
                        gv, gst.rearrange("g (p s) -> g s p", s=2),
                        mybir.AxisListType.X)
                    nc.vector.tensor_scalar_mul(gv, gv, 1.0 / gsize)
                    std = gns.tile([G, 1], F32, name="std", tag="std")
                    # mu^2 - E[x^2] = -var
                    nc.vector.scalar_tensor_tensor(
                        std, in0=gv[:, 0:1], scalar=gv[:, 0:1], in1=gv[:, 1:2],
                        op0=AL.mult, op1=AL.subtract)
                    # var + eps
                    nc.vector.tensor_scalar(std, std, -1.0, EPS,
                                            op0=AL.mult, op1=AL.add)
                    nc.scalar.activation(std, std, AF.Sqrt)
                    rhsb = gns.tile([G, 2], F32, name="rhsb", tag="rhsb")
                    nc.vector.reciprocal(rhsb[:, 0:1], std)
                    nc.vector.tensor_copy(rhsb[:, 1:2], gv[:, 0:1])
                    # broadcast (rstd, mu) back to per-channel: write expanded
                    # [C, 2] scratch (replicate each group row 16x via 0-step
                    # free dim on the SBUF read side), then plain reload
                    gscr2 = gnd.tile([C, 2], F32, name="gscr2", tag="gscr2")
                    rep_src = bass.AP(tensor=rhsb.tensor, offset=rhsb.offset,
                                      ap=[list(rhsb.ap[0]), [0, gsize],
                                          list(rhsb.ap[1])])
                    nc.gpsimd.dma_start(gscr2, rep_src)
                    for ci in range(NCT):
                        bc = gns.tile([128, 2], F32, name="bc", tag="bc_sb")
                        nc.gpsimd.dma_start(bc, gscr2[ci * 128:(ci + 1) * 128, :])
                        gnw_sb = gns.tile([128, 1], F32, name="gnw_sb", tag="gnw")
                        nc.gpsimd.dma_start(gnw_sb, gnw_d[ci * 128:(ci + 1) * 128, :])
                        gnb_sb = gns.tile([128, 1], F32, name="gnb_sb", tag="gnb")
                        nc.gpsimd.dma_start(gnb_sb, gnb_d[ci * 128:(ci + 1) * 128, :])
                        # A = gn_w * rstd ; B = gn_b - mu * A
                        nc.vector.tensor_mul(A_sb[ci], gnw_sb, bc[:, 0:1])
                        tmp = gns.tile([128, 1], F32, name="gn_tmp", tag="tmp")
                        nc.vector.tensor_mul(tmp, bc[:, 1:2], A_sb[ci])
                        nc.vector.tensor_sub(B_sb[ci], gnb_sb, tmp)
                    # pass B: h = A*x + B (f32 -> bf16)
                    for ci in range(NCT):
                        xt = px.tile([128, T], F32, name="gn_x2", tag="x")
                        nc.gpsimd.dma_start(xt, x_d[ci * 128:(ci + 1) * 128, :])
                        nc.vector.tensor_scalar(h_sb[ci], xt, A_sb[ci], B_sb[ci],
                                                op0=AL.mult, op1=AL.add)

                # ================= phase 2: qkv =================
                with (
                    tc.tile_pool(name="qkvb", bufs=1) as qkvb,
                    tc.tile_pool(name="qkvp", bufs=3, space="PSUM") as qkvp,
                ):
                    bq_sb, bk_sb = [], []
                    for i in range(NCT):
                        b = qkvb.tile([128, 1], F32, name=f"bq{i}", tag=f"bq{i}")
                        nc.gpsimd.dma_start(b, bqk_d[i * 128:(i + 1) * 128, :])
                        bq_sb.append(b)
                    for i in range(NCT):
                        b = qkvb.tile([128, 1], F32, name=f"bk{i}", tag=f"bk{i}")
                        nc.gpsimd.dma_start(b, bqk_d[C + i * 128:C + (i + 1) * 128, :])
                        bk_sb.append(b)
                    # q then k: evacuate PSUM via scalar engine (+bias)
                    for which, dst, bias, coff in (("q", q_sb, bq_sb, 0),
                                                   ("k", k_sb, bk_sb, C)):
                        for ci in range(NCT):
                            for ch in range(NCH):
                                ps = qkvp.tile([128, TCH], F32,
                                               name="qkv_ps", tag="ps")
                                for cj in range(NCT):
                                    nc.tensor.matmul(
                                        ps,
                                        lhsT=wqkvt_sb[cj][:, coff + ci * 128:
                                                          coff + (ci + 1) * 128],
                                        rhs=h_sb[cj][:, ch * TCH:(ch + 1) * TCH],
                                        start=(cj == 0), stop=(cj == NCT - 1))
                                nc.scalar.activation(
                                    dst[ci][:, ch * TCH:(ch + 1) * TCH], ps,
                                    AF.Identity, bias=bias[ci])
                    # vT (v bias folded into bout on host)
                    for st in range(NT):
                        ps = qkvp.tile([128, C], F32, name="qkv_ps2", tag="ps")
                        for cj in range(NCT):
                            nc.tensor.matmul(
                                ps, lhsT=h_sb[cj][:, st * 128:(st + 1) * 128],
                                rhs=wqkvt_sb[cj][:, 2 * C:3 * C],
                                start=(cj == 0), stop=(cj == NCT - 1))
                        nc.vector.tensor_copy(vT_sb[st], ps)

            # ================= phase 3: attention + proj =================
            with (
                tc.tile_pool(name="p3s", bufs=1) as p3s,
                tc.tile_pool(name="p3w", bufs=2) as p3w,
                tc.tile_pool(name="p3d", bufs=2, space="DRAM") as p3d,
                tc.tile_pool(name="pst", bufs=2, space="PSUM") as pst,
                tc.tile_pool(name="pden", bufs=2, space="PSUM") as pden,
                tc.tile_pool(name="ppv", bufs=2, space="PSUM") as ppv,
                tc.tile_pool(name="ppr", bufs=2, space="PSUM") as ppr,
            ):
                for tci in range(NCH):
                    t0 = tci * TCH
                    # early residual x reload (overlaps compute)
                    xr = []
                    for oi in range(NCT):
                        xt = p3w.tile([128, TCH], F32, name=f"xr{oi}",
                                      tag=f"xr{oi}")
                        nc.gpsimd.dma_start(xt, x_d[oi * 128:(oi + 1) * 128,
                                                  t0:t0 + TCH])
                        xr.append(xt)
                    # scores^T + exp -> PT tiles [s,t] in SBUF (bf16)
                    pt = p3s.tile([128, NT, TCH], BF16, name="pt", tag="pt")
                    for st in range(NT):
                        sp = pst.tile([128, TCH], F32, name="st_ps", tag="st")
                        for cj in range(NCT):
                            nc.tensor.matmul(
                                sp, lhsT=k_sb[cj][:, st * 128:(st + 1) * 128],
                                rhs=q_sb[cj][:, t0:t0 + TCH],
                                start=(cj == 0), stop=(cj == NCT - 1))
                        nc.scalar.activation(pt[:, st, :], sp, AF.Exp)
                    # softmax denominators: ones^T @ PT, then reciprocal
                    dps = pden.tile([1, TCH], F32, name="den_ps", tag="den")
                    for st in range(NT):
                        nc.tensor.matmul(dps, lhsT=ones_sb, rhs=pt[:, st, :],
                                         start=(st == 0), stop=(st == NT - 1))
                    den = p3w.tile([1, TCH], F32, name="den_sb", tag="den_sb")
                    nc.vector.reciprocal(den, dps)
                    # broadcast denominators across partitions via DRAM
                    dscr = p3d.tile([1, TCH], F32, name="dscr", tag="dscr")
                    nc.gpsimd.dma_start(dscr, den)
                    den_bc = p3w.tile([128, TCH], F32, name="den_bc", tag="den_bc")
                    dsrc = bass.AP(tensor=dscr.tensor, offset=dscr.offset,
                                   ap=[[0, 128], [1, TCH]])
                    nc.gpsimd.dma_start(den_bc, dsrc)
                    # PV
                    attn = []
                    for ci in range(NCT):
                        pv = ppv.tile([128, TCH], F32, name="pv_ps", tag="pv")
                        for st in range(NT):
                            nc.tensor.matmul(
                                pv, lhsT=vT_sb[st][:, ci * 128:(ci + 1) * 128],
                                rhs=pt[:, st, :],
                                start=(st == 0), stop=(st == NT - 1))
                        at = p3w.tile([128, TCH], BF16, name=f"attn{ci}",
                                      tag=f"attn{ci}")
                        nc.vector.tensor_mul(at, pv, den_bc)
                        attn.append(at)
                    # proj + bias + residual
                    for oi in range(NCT):
                        pr = ppr.tile([128, TCH], F32, name="pr_ps", tag="pr")
                        for cj in range(NCT):
                            nc.tensor.matmul(
                                pr, lhsT=wprojt_sb[cj][:, oi * 128:(oi + 1) * 128],
                                rhs=attn[cj],
                                start=(cj == 0), stop=(cj == NCT - 1))
                        osb = p3w.tile([128, TCH], F32, name="osb", tag="osb",
                                       bufs=3)
                        nc.vector.scalar_tensor_tensor(
                            osb, in0=pr, scalar=bout_sb[oi], in1=xr[oi],
                            op0=AL.add, op1=AL.add)
                        nc.gpsimd.dma_start(
                            out_d[oi * 128:(oi + 1) * 128, t0:t0 + TCH], osb)
    _install_wait_legalizer(nc)
    return nc


def host_prep(gn_w, gn_b, qkv_w, qkv_b, proj_w, proj_b):
    """One-time O(C^2) weight prep in numpy -> per-core replicated inputs."""
    scale = float(C) ** -0.25
    wq, wk, wv = qkv_w[:C], qkv_w[C:2 * C], qkv_w[2 * C:]
    wqkvt = np.concatenate(
        [wq.T * scale, wk.T * scale, wv.T], axis=1).astype(ml_dtypes.bfloat16)
    wprojt = np.ascontiguousarray(proj_w.T).astype(ml_dtypes.bfloat16)
    bqk = (qkv_b[:2 * C] * scale).astype(np.float32).reshape(2 * C, 1)
    bout = (proj_w @ qkv_b[2 * C:] + proj_b).astype(np.float32).reshape(C, 1)
    return {
        "wqkvt": wqkvt, "wprojt": wprojt, "bqk": bqk, "bout": bout,
        "gnw": gn_w.astype(np.float32).reshape(C, 1),
        "gnb": gn_b.astype(np.float32).reshape(C, 1),
    }


_graph_cache = {}


def run(x, gn_w, gn_b, qkv_w, qkv_b, proj_w, proj_b, trace=False):
    x = np.asarray(x, np.float32)
    B, Cv, H, W = x.shape
    T = H * W
    shared = host_prep(np.asarray(gn_w), np.asarray(gn_b),
                       np.asarray(qkv_w), np.asarray(qkv_b),
                       np.asarray(proj_w), np.asarray(proj_b))
    key = (T, B)
    if key not in _graph_cache:
        _graph_cache[key] = build_graph(T, n_cores=B)
    nc = _graph_cache[key]
    in_maps = []
    for i in range(B):
        m = dict(shared)
        m["x"] = np.ascontiguousarray(x[i].reshape(Cv, T))
        in_maps.append(m)
    try:
        res = run_bass_kernel_spmd(nc, in_maps, core_ids=list(range(B)),
                                   trace=trace)
    except ModuleNotFoundError:
        # axon NTFF profiling hook unavailable in this container
        res = run_bass_kernel_spmd(nc, in_maps, core_ids=list(range(B)),
                                   trace=False)
    out = np.stack([res.results[i]["out"] for i in range(B)])
    return out.reshape(B, Cv, H, W).astype(np.float32), res


def kernel(**inputs):
    out, _ = run(**inputs)
    return out

